# revision 40
# baseline (speedup 1.0000x reference)
"""Trainium2 Bass kernel for the EnergyCoulomb problem.

Reference computation (per molecule, B=32, N=512, D=1024, H=512):
  y  = sum_atoms(mask * (ssp(rep @ W1 + b1) @ W2 + b2))           atomwise MLP + pool
  q  = ssp(rep @ Wc1 + bc1) @ Wc2 + bc2                           charge net
  e  = sum_{i!=j} q_i q_j (1e-5 + |R_i - R_j|)^-2 * mask_i mask_j coulomb term
  out = y + e

Sharding: data-parallel over molecules, 4 molecules per core on 8 cores,
weights replicated. Per core everything is computed transposed
(z^T = W^T @ rep^T) so biases are per-partition and the second-layer
contraction over H runs on the PE with h on partitions.

ssp(x) = softplus(x) - ln2 is folded as softplus on device plus a host-side
constant shift c = b - ln2 * sum(W_layer2) applied at the pooled level.

Pairwise distances are computed exactly (per coordinate broadcast-subtract +
square on ACT, accumulate on DVE) rather than via the |ri|^2+|rj|^2-2ri.rj
matmul trick, which loses ~3 decimal digits to cancellation for close pairs.
"""

import numpy as np

import concourse.bass as bass
import concourse.bacc as bacc
import concourse.mybir as mybir
import concourse.tile as tile
from concourse import bass_utils
from concourse.masks import make_identity

LOG2 = float(np.log(2.0))

B, N, D, H = 32, 512, 1024, 512
NCORES = 8
BL = B // NCORES          # molecules per core
P = 128                   # partitions
KD = D // P               # 8 K-chunks over D
HC = H // P               # 4 h-chunks over H
IC = N // P               # 4 i-chunks over atoms

f32 = mybir.dt.float32
f32r = mybir.dt.float32r
AF = mybir.ActivationFunctionType
ALU = mybir.AluOpType
AX = mybir.AxisListType

_CACHE = {}

# Every ACT function this kernel uses (Exp, Ln, Square, Copy, Identity) lives
# in the "natural_log_exp_and_others" table set. Bacc's table chooser is
# greedy-first-match, which makes alternating Exp/Ln streams flip-flop between
# "exp_and_others" and "natural_log" — 65 table loads x ~2.7us. Emptying every
# other set (order preserved, so act_func_set_id indices stay valid) pins the
# chooser to the combined set: one load for the whole kernel.
_ONE_TABLE = "natural_log_exp_and_others"


def _gat_one_table(arch):
    from concourse.hw_specs import get_activation_tables
    tabs = get_activation_tables(arch)
    assert _ONE_TABLE in tabs
    return {n: (fns if n == _ONE_TABLE else set()) for n, fns in tabs.items()}


def _build_program():
    bacc.get_activation_tables = _gat_one_table
    nc = bacc.Bacc("TRN2", target_bir_lowering=False, debug=False,
                   enable_asserts=False)

    rep_d = nc.dram_tensor("rep", [BL * N, D], f32r, kind="ExternalInput").ap()
    w1_d = nc.dram_tensor("w1", [D, H], f32r, kind="ExternalInput").ap()
    wc1_d = nc.dram_tensor("wc1", [D, H], f32r, kind="ExternalInput").ap()
    b1t_d = nc.dram_tensor("b1t", [P, HC], f32, kind="ExternalInput").ap()
    bc1t_d = nc.dram_tensor("bc1t", [P, HC], f32, kind="ExternalInput").ap()
    w2t_d = nc.dram_tensor("w2t", [P, HC], f32r, kind="ExternalInput").ap()
    wc2t_d = nc.dram_tensor("wc2t", [P, HC], f32r, kind="ExternalInput").ap()
    rrows_d = nc.dram_tensor("rrows", [BL, 3, N], f32, kind="ExternalInput").ap()
    rcoln_d = nc.dram_tensor("rcoln", [P, BL * IC * 3], f32, kind="ExternalInput").ap()
    maskr_d = nc.dram_tensor("maskr", [BL, N], f32, kind="ExternalInput").ap()
    cvec_d = nc.dram_tensor("cvec", [1, BL + 1], f32, kind="ExternalInput").ap()
    out_d = nc.dram_tensor("out", [1, BL], f32, kind="ExternalOutput").ap()

    with tile.TileContext(nc) as tc:
        with tc.tile_pool(name="singles", bufs=1) as singles, \
             tc.tile_pool(name="work", bufs=1) as work, \
             tc.tile_pool(name="ps", bufs=1, space="PSUM") as ps:

            ident = singles.tile([P, P], f32, tag="ident")
            make_identity(nc, ident)
            identr = singles.tile([P, P], f32r, tag="identr")
            nc.vector.tensor_copy(identr, ident)
            ident32 = singles.tile([1, 1], f32, tag="ident32")
            nc.vector.memset(ident32, 1.0)
            ones_col = singles.tile([P, 1], f32, tag="ones_col")
            nc.vector.memset(ones_col, 1.0)
            floor_col = singles.tile([P, 1], f32, tag="floor_col")
            nc.vector.memset(floor_col, 1e-30)

            # rep tiles stream on both HWDGE rings (SP + ACT), issued first so
            # the PE transposes can start a few us in; everything else rides
            # the eight SWDGE queues (gpsimd) concurrently.
            rcoln = singles.tile([P, BL * IC * 3], f32, tag="rcoln")
            nc.sync.dma_start(rcoln, rcoln_d)
            xjb0 = work.tile([P, 3, N], f32, tag="xjb", bufs=2)
            nc.sync.dma_start(xjb0, rrows_d[0].partition_broadcast(P))
            repm_all = []

            def load_rep(b):
                for mb in range(IC):
                    t = work.tile([P, D], f32r, tag="repm", bufs=2 * IC)
                    eng = nc.sync if (mb % 2 == 0) else nc.scalar
                    eng.dma_start(
                        t, rep_d[b * N + mb * P: b * N + (mb + 1) * P, :])
                    repm_all.append(t)

            # molecule-0 rep first, then the first two weight K-chunks on the
            # fast rings (the first z matmuls need them), then the rest
            load_rep(0)
            w1_sb = [None] * KD
            wc1_sb = [None] * KD
            for k in range(KD):
                w1_sb[k] = singles.tile([P, H], f32r, tag=f"w1_{k}", name=f"w1sb{k}")
                wc1_sb[k] = singles.tile([P, H], f32r, tag=f"wc1_{k}", name=f"wc1sb{k}")
            for k in range(KD):
                eng = nc.sync if k % 2 == 0 else nc.scalar
                eng.dma_start(w1_sb[k], w1_d[k * P:(k + 1) * P, :])
            nc.scalar.dma_start(wc1_sb[0], wc1_d[0:P, :])
            nc.sync.dma_start(wc1_sb[1], wc1_d[P:2 * P, :])
            for b in range(1, BL):
                load_rep(b)
            for k in range(2, KD):
                nc.gpsimd.dma_start(wc1_sb[k], wc1_d[k * P:(k + 1) * P, :])

            b1t = singles.tile([P, HC], f32, tag="b1t")
            nc.gpsimd.dma_start(b1t, b1t_d)
            bc1t = singles.tile([P, HC], f32, tag="bc1t")
            nc.gpsimd.dma_start(bc1t, bc1t_d)
            w2t = singles.tile([P, HC], f32r, tag="w2t")
            nc.gpsimd.dma_start(w2t, w2t_d)
            wc2t = singles.tile([P, HC], f32r, tag="wc2t")
            nc.gpsimd.dma_start(wc2t, wc2t_d)
            cvec = singles.tile([1, BL + 1], f32, tag="cvec")
            nc.gpsimd.dma_start(cvec, cvec_d)
            mrows = []
            for b in range(BL):
                m = singles.tile([1, N], f32, tag=f"mrow_{b}")
                nc.gpsimd.dma_start(m, maskr_d[b:b + 1, :])
                mrows.append(m)
            res = singles.tile([1, BL], f32, tag="res")

            # ---- pairwise chain: rb[p, ic, j] = (1e-5 + d_(128ic+p),j)^-2 ----
            # All four 128-row i-chunks are packed in one [P, IC, N] tile so
            # each elementwise stage is a single large instruction.
            # d2[p, ic, j] = |R_(128*ic+p) - R_j|^2, built from one ACT Square
            # per coordinate per chunk (bias = -coord_i), then
            # r = (1e-5 + sqrt(d2))^-2 with sqrt = exp(0.5*ln(d2)) to stay on
            # the single exp/ln ACT table set (a table switch costs ~2.7us).
            # d2 is exactly 0 on each chunk's diagonal; the 1e-30 ln floor
            # keeps it finite there (off-diagonal d2 >= ~1e-4 so the floor is
            # invisible) and the fused affine_select zeroes all diagonals.
            # The chain has no PE work and no dependency on the same
            # molecule's MLP, so it is emitted one molecule AHEAD: it fills
            # ACT/DVE/GpSimd while the PE grinds the previous molecule's
            # matmuls, and rb is ready when the t-matvec needs it.
            def emit_chain(b):
                if b == 0:
                    xjb = xjb0
                else:
                    xjb = work.tile([P, 3, N], f32, tag="xjb", bufs=2)
                    nc.gpsimd.dma_start(xjb, rrows_d[b].partition_broadcast(P))
                d2b = work.tile([P, IC, N], f32, tag="d2b", bufs=1)
                tmpb = work.tile([P, IC, N], f32, tag="tmpb", bufs=1)
                for ic in range(IC):
                    col = (b * IC + ic) * 3
                    nc.scalar.activation(d2b[:, ic, :], xjb[:, 0, :], AF.Square,
                                         bias=rcoln[:, col + 0:col + 1])
                    nc.scalar.activation(tmpb[:, ic, :], xjb[:, 1, :], AF.Square,
                                         bias=rcoln[:, col + 1:col + 2])
                add_eng = nc.gpsimd if b == 0 else nc.vector
                add_eng.tensor_tensor(d2b, d2b, tmpb, op=ALU.add)
                for ic in range(IC):
                    col = (b * IC + ic) * 3
                    nc.scalar.activation(tmpb[:, ic, :], xjb[:, 2, :], AF.Square,
                                         bias=rcoln[:, col + 2:col + 3])
                add_eng.tensor_tensor(d2b, d2b, tmpb, op=ALU.add)
                nc.scalar.activation(d2b, d2b, AF.Ln, bias=floor_col[:, 0:1])
                nc.scalar.activation(d2b, d2b, AF.Exp, scale=0.5)
                nc.gpsimd.tensor_scalar(d2b, d2b, 1e-5, None, op0=ALU.add)
                rcb = work.tile([P, IC, N], f32, tag="rcb", bufs=1)
                nc.vector.reciprocal(rcb, d2b)
                rb = work.tile([P, IC, N], f32r, tag="rb", bufs=3)
                nc.vector.tensor_mul(rb, rcb, rcb)
                # zero the diagonal of every chunk: j == p + 128*ic
                nc.gpsimd.affine_select(
                    out=rb, in_=rb, compare_op=ALU.not_equal, fill=0.0,
                    base=0, pattern=[[P, IC], [-1, N]], channel_multiplier=1)
                return rb

            rb_tiles = {0: emit_chain(0), 1: emit_chain(1)}

            for b in range(BL):
                repm = repm_all[b * IC:(b + 1) * IC]
                rT = []
                for db in range(KD):
                    tp = ps.tile([P, N], f32r, tag="tp", bufs=3)
                    for mb in range(IC):
                        nc.tensor.transpose(
                            tp[:, mb * P:(mb + 1) * P],
                            repm[mb][:, db * P:(db + 1) * P], identr)
                    rt = work.tile([P, N], f32r, tag="rT", bufs=12)
                    nc.vector.tensor_copy(rt, tp)
                    rT.append(rt)

                # ---- MLP layer 1 (transposed): h^T = ssp(W^T @ rep^T + b) ----
                h1T = []
                hqT = []
                for (w_sb, bias, hlist, htag) in (
                        (w1_sb, b1t, h1T, "h1"), (wc1_sb, bc1t, hqT, "hq")):
                    for hc in range(HC):
                        z = ps.tile([P, N], f32, tag="z", bufs=2)
                        for k in range(KD):
                            nc.tensor.matmul(
                                z,
                                lhsT=w_sb[k][:, hc * P:(hc + 1) * P],
                                rhs=rT[k][:],
                                start=(k == 0), stop=(k == KD - 1))
                        # softplus(z + b) = ln(exp(z + b) + 1); Softplus has
                        # no ACT table on this compiler, exp/ln share one set
                        ez = work.tile([P, N], f32, tag="ez", bufs=2)
                        nc.scalar.activation(ez, z, AF.Exp,
                                             bias=bias[:, hc:hc + 1])
                        h = work.tile([P, N], f32r, tag=htag, bufs=2 * HC)
                        nc.scalar.activation(h, ez, AF.Ln, bias=ones_col[:, 0:1])
                        hlist.append(h)

                # prefetch a later molecule's pairwise chain (ACT/DVE/GpSimd
                # only) while this molecule's PE work continues below
                if b + 2 < BL:
                    rb_tiles[b + 2] = emit_chain(b + 2)

                # ---- layer 2 contractions over H on the PE ----
                yi_ps = ps.tile([1, N], f32, tag="row_ps", bufs=2)
                for hc in range(HC):
                    nc.tensor.matmul(yi_ps,
                                     lhsT=w2t[:, hc:hc + 1],
                                     rhs=h1T[hc][:],
                                     start=(hc == 0), stop=(hc == HC - 1))
                q_ps = ps.tile([1, N], f32, tag="row_ps", bufs=2)
                for hc in range(HC):
                    nc.tensor.matmul(q_ps,
                                     lhsT=wc2t[:, hc:hc + 1],
                                     rhs=hqT[hc][:],
                                     start=(hc == 0), stop=(hc == HC - 1))

                mrow = mrows[b]
                # y_b = sum(yi * mask) + cm_b, with cm_b = c2*sum(mask_b)
                # precomputed on host (cvec[0, b]).
                # tensor_tensor_reduce with a PSUM operand faults the exec
                # unit on this runtime; use mul + reduce instead
                scr_y = work.tile([1, N], f32, tag="scr_y", bufs=2)
                nc.vector.tensor_mul(scr_y, yi_ps, mrow)
                ysum = work.tile([1, 1], f32, tag="ysum", bufs=2)
                nc.vector.reduce_sum(ysum, scr_y, axis=AX.X)
                y_sb = work.tile([1, 1], f32, tag="y_sb", bufs=2)
                nc.vector.tensor_add(y_sb, ysum, cvec[0:1, b:b + 1])

                # charge row: qrow = (q + cq) * mask
                qrow = work.tile([1, N], f32, tag="qrow", bufs=2)
                nc.vector.tensor_scalar(qrow, q_ps, cvec[0:1, BL:BL + 1], None,
                                        op0=ALU.add)
                nc.vector.tensor_mul(qrow, qrow, mrow)

                # charge columns (one [128,1] per i-chunk) via PE transpose
                qc_ps = ps.tile([P, IC], f32, tag="qc_ps", bufs=1)
                for ic in range(IC):
                    nc.tensor.transpose(qc_ps[:, ic:ic + 1],
                                        qrow[:, ic * P:(ic + 1) * P],
                                        ident32[0:1, 0:1])
                qc = work.tile([P, IC], f32r, tag="qc", bufs=2)
                nc.scalar.copy(qc, qc_ps)

                rb = rb_tiles.pop(b)
                t_ps = ps.tile([1, N], f32, tag="row_ps", bufs=2)
                for ic in range(IC):
                    nc.tensor.matmul(t_ps,
                                     lhsT=qc[:, ic:ic + 1],
                                     rhs=rb[:, ic, :],
                                     start=(ic == 0), stop=(ic == IC - 1))

                scr_e = work.tile([1, N], f32, tag="scr_e", bufs=2)
                nc.vector.tensor_mul(scr_e, t_ps, qrow)
                e_sb = work.tile([1, 1], f32, tag="e_sb", bufs=2)
                nc.vector.reduce_sum(e_sb, scr_e, axis=AX.X)

                nc.vector.tensor_add(res[:, b:b + 1], y_sb, e_sb)

            nc.sync.dma_start(out_d, res)

    nc.compile()
    return nc


def _get_program():
    if "nc" not in _CACHE:
        _CACHE["nc"] = _build_program()
    return _CACHE["nc"]


def _host_prep(inputs):
    """Build per-core in_maps from full inputs."""
    rep = np.ascontiguousarray(np.asarray(inputs["representation"], np.float32))
    R = np.asarray(inputs["R"], np.float32)
    mask = np.asarray(inputs["atom_mask"], np.float32)
    W1 = np.asarray(inputs["W1"], np.float32)
    b1 = np.asarray(inputs["b1"], np.float32)
    W2 = np.asarray(inputs["W2"], np.float32)
    b2 = np.asarray(inputs["b2"], np.float32)
    Wc1 = np.asarray(inputs["Wc1"], np.float32)
    bc1 = np.asarray(inputs["bc1"], np.float32)
    Wc2 = np.asarray(inputs["Wc2"], np.float32)
    bc2 = np.asarray(inputs["bc2"], np.float32)

    b1t = np.ascontiguousarray(b1.reshape(HC, P).T)
    bc1t = np.ascontiguousarray(bc1.reshape(HC, P).T)
    w2t = np.ascontiguousarray(W2[:, 0].reshape(HC, P).T)
    wc2t = np.ascontiguousarray(Wc2[:, 0].reshape(HC, P).T)
    c2 = np.float32(b2[0] - LOG2 * W2.sum(dtype=np.float64))
    cq = np.float32(bc2[0] - LOG2 * Wc2.sum(dtype=np.float64))

    in_maps = []
    for c in range(NCORES):
        sl = slice(c * BL, (c + 1) * BL)
        Rb = R[sl]                                   # [BL, N, 3]
        rrows = np.ascontiguousarray(Rb.transpose(0, 2, 1))       # [BL,3,N]
        # rcoln[p, (b*IC+ic)*3 + c] = -R[b, ic*128+p, c]
        rcoln = np.ascontiguousarray(
            (-Rb.reshape(BL, IC, P, 3)).transpose(2, 0, 1, 3).reshape(P, BL * IC * 3))
        # cvec = [c2*sum(mask_b) per molecule, then cq]
        cvec = np.concatenate(
            [c2 * mask[sl].sum(axis=1, dtype=np.float32), [cq]]
        ).astype(np.float32).reshape(1, BL + 1)
        in_maps.append({
            "rep": np.ascontiguousarray(rep[sl].reshape(BL * N, D)),
            "w1": W1, "wc1": Wc1,
            "b1t": b1t, "bc1t": bc1t, "w2t": w2t, "wc2t": wc2t,
            "rrows": rrows, "rcoln": rcoln,
            "maskr": np.ascontiguousarray(mask[sl]),
            "cvec": cvec,
        })
    return in_maps


def kernel(**inputs) -> np.ndarray:
    nc = _get_program()
    in_maps = _host_prep(inputs)
    res = None
    last_err = None
    for attempt in range(3):
        try:
            res = bass_utils.run_bass_kernel_spmd(
                nc, in_maps, core_ids=list(range(NCORES)))
            break
        except Exception as e:  # transient NRT_EXEC_UNIT faults have been seen
            last_err = e
            import time
            time.sleep(2.0)
            try:
                import jax
                jax.clear_backends()
            except Exception:
                pass
    if res is None:
        raise last_err
    out = np.concatenate([res.results[c]["out"][0] for c in range(NCORES)])
    return out.reshape(B, 1).astype(np.float32)


# revision 41
# speedup vs baseline: 1.0454x; 1.0454x over previous
"""Trainium2 Bass kernel for the EnergyCoulomb problem.

Reference computation (per molecule, B=32, N=512, D=1024, H=512):
  y  = sum_atoms(mask * (ssp(rep @ W1 + b1) @ W2 + b2))           atomwise MLP + pool
  q  = ssp(rep @ Wc1 + bc1) @ Wc2 + bc2                           charge net
  e  = sum_{i!=j} q_i q_j (1e-5 + |R_i - R_j|)^-2 * mask_i mask_j coulomb term
  out = y + e

Sharding: data-parallel over molecules, 4 molecules per core on 8 cores,
weights replicated. Per core everything is computed transposed
(z^T = W^T @ rep^T) so biases are per-partition and the second-layer
contraction over H runs on the PE with h on partitions.

ssp(x) = softplus(x) - ln2 is folded as softplus on device plus a host-side
constant shift c = b - ln2 * sum(W_layer2) applied at the pooled level.

Pairwise distances are computed exactly (per coordinate broadcast-subtract +
square on ACT, accumulate on DVE) rather than via the |ri|^2+|rj|^2-2ri.rj
matmul trick, which loses ~3 decimal digits to cancellation for close pairs.
"""

import numpy as np

import concourse.bass as bass
import concourse.bacc as bacc
import concourse.mybir as mybir
import concourse.tile as tile
from concourse import bass_utils
from concourse.masks import make_identity

LOG2 = float(np.log(2.0))

B, N, D, H = 32, 512, 1024, 512
NCORES = 8
BL = B // NCORES          # molecules per core
P = 128                   # partitions
KD = D // P               # 8 K-chunks over D
HC = H // P               # 4 h-chunks over H
IC = N // P               # 4 i-chunks over atoms

f32 = mybir.dt.float32
f32r = mybir.dt.float32r
AF = mybir.ActivationFunctionType
ALU = mybir.AluOpType
AX = mybir.AxisListType

_CACHE = {}

# Every ACT function this kernel uses (Exp, Ln, Square, Copy, Identity) lives
# in the "natural_log_exp_and_others" table set. Bacc's table chooser is
# greedy-first-match, which makes alternating Exp/Ln streams flip-flop between
# "exp_and_others" and "natural_log" — 65 table loads x ~2.7us. Emptying every
# other set (order preserved, so act_func_set_id indices stay valid) pins the
# chooser to the combined set: one load for the whole kernel.
_ONE_TABLE = "natural_log_exp_and_others"


def _gat_one_table(arch):
    from concourse.hw_specs import get_activation_tables
    tabs = get_activation_tables(arch)
    assert _ONE_TABLE in tabs
    return {n: (fns if n == _ONE_TABLE else set()) for n, fns in tabs.items()}


def _build_program():
    bacc.get_activation_tables = _gat_one_table
    nc = bacc.Bacc("TRN2", target_bir_lowering=False, debug=False,
                   enable_asserts=False)

    rep_d = nc.dram_tensor("rep", [BL * N, D], f32r, kind="ExternalInput").ap()
    w1_d = nc.dram_tensor("w1", [D, H], f32r, kind="ExternalInput").ap()
    wc1_d = nc.dram_tensor("wc1", [D, H], f32r, kind="ExternalInput").ap()
    b1t_d = nc.dram_tensor("b1t", [P, HC], f32, kind="ExternalInput").ap()
    bc1t_d = nc.dram_tensor("bc1t", [P, HC], f32, kind="ExternalInput").ap()
    w2t_d = nc.dram_tensor("w2t", [P, HC], f32r, kind="ExternalInput").ap()
    wc2t_d = nc.dram_tensor("wc2t", [P, HC], f32r, kind="ExternalInput").ap()
    rrows_d = nc.dram_tensor("rrows", [BL, 3, N], f32, kind="ExternalInput").ap()
    rcoln_d = nc.dram_tensor("rcoln", [P, BL * IC * 3], f32, kind="ExternalInput").ap()
    maskr_d = nc.dram_tensor("maskr", [BL, N], f32, kind="ExternalInput").ap()
    cvec_d = nc.dram_tensor("cvec", [1, BL + 1], f32, kind="ExternalInput").ap()
    out_d = nc.dram_tensor("out", [1, BL], f32, kind="ExternalOutput").ap()

    with tile.TileContext(nc) as tc:
        with tc.tile_pool(name="singles", bufs=1) as singles, \
             tc.tile_pool(name="work", bufs=1) as work, \
             tc.tile_pool(name="ps", bufs=1, space="PSUM") as ps:

            ident = singles.tile([P, P], f32, tag="ident")
            make_identity(nc, ident)
            identr = singles.tile([P, P], f32r, tag="identr")
            nc.vector.tensor_copy(identr, ident)
            ident32 = singles.tile([1, 1], f32, tag="ident32")
            nc.vector.memset(ident32, 1.0)
            ones_col = singles.tile([P, 1], f32, tag="ones_col")
            nc.vector.memset(ones_col, 1.0)
            floor_col = singles.tile([P, 1], f32, tag="floor_col")
            nc.vector.memset(floor_col, 1e-30)

            # rep tiles stream on both HWDGE rings (SP + ACT), issued first so
            # the PE transposes can start a few us in; everything else rides
            # the eight SWDGE queues (gpsimd) concurrently.
            rcoln = singles.tile([P, BL * IC * 3], f32, tag="rcoln")
            nc.sync.dma_start(rcoln, rcoln_d)
            xjb0 = work.tile([P, 3, N], f32, tag="xjb", bufs=2)
            nc.sync.dma_start(xjb0, rrows_d[0].partition_broadcast(P))
            repm_all = []

            def load_rep(b):
                for mb in range(IC):
                    t = work.tile([P, D], f32r, tag="repm", bufs=2 * IC)
                    eng = nc.sync if (mb % 2 == 0) else nc.scalar
                    eng.dma_start(
                        t, rep_d[b * N + mb * P: b * N + (mb + 1) * P, :])
                    repm_all.append(t)

            # molecule-0 rep first, then the first two weight K-chunks on the
            # fast rings (the first z matmuls need them), then the rest
            load_rep(0)
            w1_sb = [None] * KD
            wc1_sb = [None] * KD
            for k in range(KD):
                w1_sb[k] = singles.tile([P, H], f32r, tag=f"w1_{k}", name=f"w1sb{k}")
                wc1_sb[k] = singles.tile([P, H], f32r, tag=f"wc1_{k}", name=f"wc1sb{k}")
            for k in range(KD):
                eng = nc.sync if k % 2 == 0 else nc.scalar
                eng.dma_start(w1_sb[k], w1_d[k * P:(k + 1) * P, :])
            nc.scalar.dma_start(wc1_sb[0], wc1_d[0:P, :])
            nc.sync.dma_start(wc1_sb[1], wc1_d[P:2 * P, :])
            for b in range(1, BL):
                load_rep(b)
            for k in range(2, KD):
                nc.gpsimd.dma_start(wc1_sb[k], wc1_d[k * P:(k + 1) * P, :])

            b1t = singles.tile([P, HC], f32, tag="b1t")
            nc.gpsimd.dma_start(b1t, b1t_d)
            bc1t = singles.tile([P, HC], f32, tag="bc1t")
            nc.gpsimd.dma_start(bc1t, bc1t_d)
            w2t = singles.tile([P, HC], f32r, tag="w2t")
            nc.gpsimd.dma_start(w2t, w2t_d)
            wc2t = singles.tile([P, HC], f32r, tag="wc2t")
            nc.gpsimd.dma_start(wc2t, wc2t_d)
            cvec = singles.tile([1, BL + 1], f32, tag="cvec")
            nc.gpsimd.dma_start(cvec, cvec_d)
            mrows = []
            for b in range(BL):
                m = singles.tile([1, N], f32, tag=f"mrow_{b}")
                nc.gpsimd.dma_start(m, maskr_d[b:b + 1, :])
                mrows.append(m)
            res = singles.tile([1, BL], f32, tag="res")

            # ---- pairwise chain: rb[p, ic, j] = (1e-5 + d_(128ic+p),j)^-2 ----
            # All four 128-row i-chunks are packed in one [P, IC, N] tile so
            # each elementwise stage is a single large instruction.
            # d2[p, ic, j] = |R_(128*ic+p) - R_j|^2, built from one ACT Square
            # per coordinate per chunk (bias = -coord_i), then
            # r = (1e-5 + sqrt(d2))^-2 with sqrt = exp(0.5*ln(d2)) to stay on
            # the single exp/ln ACT table set (a table switch costs ~2.7us).
            # d2 is exactly 0 on each chunk's diagonal; the 1e-30 ln floor
            # keeps it finite there (off-diagonal d2 >= ~1e-4 so the floor is
            # invisible) and the fused affine_select zeroes all diagonals.
            # The chain has no PE work and no dependency on the same
            # molecule's MLP, so it is emitted one molecule AHEAD: it fills
            # ACT/DVE/GpSimd while the PE grinds the previous molecule's
            # matmuls, and rb is ready when the t-matvec needs it.
            def emit_chain(b):
                if b == 0:
                    xjb = xjb0
                else:
                    xjb = work.tile([P, 3, N], f32, tag="xjb", bufs=2)
                    nc.gpsimd.dma_start(xjb, rrows_d[b].partition_broadcast(P))
                d2b = work.tile([P, IC, N], f32, tag="d2b", bufs=1)
                tmpb = work.tile([P, IC, N], f32, tag="tmpb", bufs=1)
                for ic in range(IC):
                    col = (b * IC + ic) * 3
                    nc.scalar.activation(d2b[:, ic, :], xjb[:, 0, :], AF.Square,
                                         bias=rcoln[:, col + 0:col + 1])
                    nc.scalar.activation(tmpb[:, ic, :], xjb[:, 1, :], AF.Square,
                                         bias=rcoln[:, col + 1:col + 2])
                add_eng = nc.gpsimd if b == 0 else nc.vector
                add_eng.tensor_tensor(d2b, d2b, tmpb, op=ALU.add)
                for ic in range(IC):
                    col = (b * IC + ic) * 3
                    nc.scalar.activation(tmpb[:, ic, :], xjb[:, 2, :], AF.Square,
                                         bias=rcoln[:, col + 2:col + 3])
                add_eng.tensor_tensor(d2b, d2b, tmpb, op=ALU.add)
                nc.scalar.activation(d2b, d2b, AF.Ln, bias=floor_col[:, 0:1])
                nc.scalar.activation(d2b, d2b, AF.Exp, scale=0.5)
                nc.gpsimd.tensor_scalar(d2b, d2b, 1e-5, None, op0=ALU.add)
                rcb = work.tile([P, IC, N], f32, tag="rcb", bufs=1)
                nc.vector.reciprocal(rcb, d2b)
                rb = work.tile([P, IC, N], f32r, tag="rb", bufs=2)
                nc.vector.tensor_mul(rb, rcb, rcb)
                # zero the diagonal of every chunk: j == p + 128*ic
                nc.gpsimd.affine_select(
                    out=rb, in_=rb, compare_op=ALU.not_equal, fill=0.0,
                    base=0, pattern=[[P, IC], [-1, N]], channel_multiplier=1)
                return rb

            rb_tiles = {0: emit_chain(0), 1: emit_chain(1)}

            for b in range(BL):
                repm = repm_all[b * IC:(b + 1) * IC]
                rT = []
                for db in range(KD):
                    tp = ps.tile([P, N], f32r, tag="tp", bufs=3)
                    for mb in range(IC):
                        nc.tensor.transpose(
                            tp[:, mb * P:(mb + 1) * P],
                            repm[mb][:, db * P:(db + 1) * P], identr)
                    rt = work.tile([P, N], f32r, tag="rT", bufs=12)
                    nc.vector.tensor_copy(rt, tp)
                    rT.append(rt)

                # ---- MLP layer 1 (transposed): h^T = ssp(W^T @ rep^T + b) ----
                h1T = []
                hqT = []
                for (w_sb, bias, hlist, htag) in (
                        (w1_sb, b1t, h1T, "h1"), (wc1_sb, bc1t, hqT, "hq")):
                    for hc in range(HC):
                        z = ps.tile([P, N], f32, tag="z", bufs=2)
                        for k in range(KD):
                            nc.tensor.matmul(
                                z,
                                lhsT=w_sb[k][:, hc * P:(hc + 1) * P],
                                rhs=rT[k][:],
                                start=(k == 0), stop=(k == KD - 1))
                        # softplus(z + b) = ln(exp(z + b) + 1); Softplus has
                        # no ACT table on this compiler, exp/ln share one set
                        ez = work.tile([P, N], f32, tag="ez", bufs=2)
                        nc.scalar.activation(ez, z, AF.Exp,
                                             bias=bias[:, hc:hc + 1])
                        h = work.tile([P, N], f32r, tag=htag, bufs=2 * HC)
                        nc.scalar.activation(h, ez, AF.Ln, bias=ones_col[:, 0:1])
                        hlist.append(h)

                # prefetch a later molecule's pairwise chain (ACT/DVE/GpSimd
                # only) while this molecule's PE work continues below
                if b >= 1 and b + 1 < BL:
                    rb_tiles[b + 1] = emit_chain(b + 1)

                # ---- layer 2 contractions over H on the PE ----
                yi_ps = ps.tile([1, N], f32, tag="row_ps", bufs=2)
                for hc in range(HC):
                    nc.tensor.matmul(yi_ps,
                                     lhsT=w2t[:, hc:hc + 1],
                                     rhs=h1T[hc][:],
                                     start=(hc == 0), stop=(hc == HC - 1))
                q_ps = ps.tile([1, N], f32, tag="row_ps", bufs=2)
                for hc in range(HC):
                    nc.tensor.matmul(q_ps,
                                     lhsT=wc2t[:, hc:hc + 1],
                                     rhs=hqT[hc][:],
                                     start=(hc == 0), stop=(hc == HC - 1))

                mrow = mrows[b]
                # y_b = sum(yi * mask) + cm_b, with cm_b = c2*sum(mask_b)
                # precomputed on host (cvec[0, b]).
                # tensor_tensor_reduce with a PSUM operand faults the exec
                # unit on this runtime; use mul + reduce instead
                scr_y = work.tile([1, N], f32, tag="scr_y", bufs=2)
                nc.vector.tensor_mul(scr_y, yi_ps, mrow)
                ysum = work.tile([1, 1], f32, tag="ysum", bufs=2)
                nc.vector.reduce_sum(ysum, scr_y, axis=AX.X)
                y_sb = work.tile([1, 1], f32, tag="y_sb", bufs=2)
                nc.vector.tensor_add(y_sb, ysum, cvec[0:1, b:b + 1])

                # charge row: qrow = (q + cq) * mask
                qrow = work.tile([1, N], f32, tag="qrow", bufs=2)
                nc.vector.tensor_scalar(qrow, q_ps, cvec[0:1, BL:BL + 1], None,
                                        op0=ALU.add)
                nc.vector.tensor_mul(qrow, qrow, mrow)

                # charge columns (one [128,1] per i-chunk) via PE transpose
                qc_ps = ps.tile([P, IC], f32, tag="qc_ps", bufs=1)
                for ic in range(IC):
                    nc.tensor.transpose(qc_ps[:, ic:ic + 1],
                                        qrow[:, ic * P:(ic + 1) * P],
                                        ident32[0:1, 0:1])
                qc = work.tile([P, IC], f32r, tag="qc", bufs=2)
                nc.scalar.copy(qc, qc_ps)

                rb = rb_tiles.pop(b)
                t_ps = ps.tile([1, N], f32, tag="row_ps", bufs=2)
                for ic in range(IC):
                    nc.tensor.matmul(t_ps,
                                     lhsT=qc[:, ic:ic + 1],
                                     rhs=rb[:, ic, :],
                                     start=(ic == 0), stop=(ic == IC - 1))

                scr_e = work.tile([1, N], f32, tag="scr_e", bufs=2)
                nc.vector.tensor_mul(scr_e, t_ps, qrow)
                e_sb = work.tile([1, 1], f32, tag="e_sb", bufs=2)
                nc.vector.reduce_sum(e_sb, scr_e, axis=AX.X)

                nc.vector.tensor_add(res[:, b:b + 1], y_sb, e_sb)

            nc.sync.dma_start(out_d, res)

    nc.compile()
    return nc


def _get_program():
    if "nc" not in _CACHE:
        _CACHE["nc"] = _build_program()
    return _CACHE["nc"]


def _host_prep(inputs):
    """Build per-core in_maps from full inputs."""
    rep = np.ascontiguousarray(np.asarray(inputs["representation"], np.float32))
    R = np.asarray(inputs["R"], np.float32)
    mask = np.asarray(inputs["atom_mask"], np.float32)
    W1 = np.asarray(inputs["W1"], np.float32)
    b1 = np.asarray(inputs["b1"], np.float32)
    W2 = np.asarray(inputs["W2"], np.float32)
    b2 = np.asarray(inputs["b2"], np.float32)
    Wc1 = np.asarray(inputs["Wc1"], np.float32)
    bc1 = np.asarray(inputs["bc1"], np.float32)
    Wc2 = np.asarray(inputs["Wc2"], np.float32)
    bc2 = np.asarray(inputs["bc2"], np.float32)

    b1t = np.ascontiguousarray(b1.reshape(HC, P).T)
    bc1t = np.ascontiguousarray(bc1.reshape(HC, P).T)
    w2t = np.ascontiguousarray(W2[:, 0].reshape(HC, P).T)
    wc2t = np.ascontiguousarray(Wc2[:, 0].reshape(HC, P).T)
    c2 = np.float32(b2[0] - LOG2 * W2.sum(dtype=np.float64))
    cq = np.float32(bc2[0] - LOG2 * Wc2.sum(dtype=np.float64))

    in_maps = []
    for c in range(NCORES):
        sl = slice(c * BL, (c + 1) * BL)
        Rb = R[sl]                                   # [BL, N, 3]
        rrows = np.ascontiguousarray(Rb.transpose(0, 2, 1))       # [BL,3,N]
        # rcoln[p, (b*IC+ic)*3 + c] = -R[b, ic*128+p, c]
        rcoln = np.ascontiguousarray(
            (-Rb.reshape(BL, IC, P, 3)).transpose(2, 0, 1, 3).reshape(P, BL * IC * 3))
        # cvec = [c2*sum(mask_b) per molecule, then cq]
        cvec = np.concatenate(
            [c2 * mask[sl].sum(axis=1, dtype=np.float32), [cq]]
        ).astype(np.float32).reshape(1, BL + 1)
        in_maps.append({
            "rep": np.ascontiguousarray(rep[sl].reshape(BL * N, D)),
            "w1": W1, "wc1": Wc1,
            "b1t": b1t, "bc1t": bc1t, "w2t": w2t, "wc2t": wc2t,
            "rrows": rrows, "rcoln": rcoln,
            "maskr": np.ascontiguousarray(mask[sl]),
            "cvec": cvec,
        })
    return in_maps


def kernel(**inputs) -> np.ndarray:
    nc = _get_program()
    in_maps = _host_prep(inputs)
    res = None
    last_err = None
    for attempt in range(3):
        try:
            res = bass_utils.run_bass_kernel_spmd(
                nc, in_maps, core_ids=list(range(NCORES)))
            break
        except Exception as e:  # transient NRT_EXEC_UNIT faults have been seen
            last_err = e
            import time
            time.sleep(2.0)
            try:
                import jax
                jax.clear_backends()
            except Exception:
                pass
    if res is None:
        raise last_err
    out = np.concatenate([res.results[c]["out"][0] for c in range(NCORES)])
    return out.reshape(B, 1).astype(np.float32)


# revision 42
# speedup vs baseline: 1.0691x; 1.0226x over previous
"""Trainium2 Bass kernel for the EnergyCoulomb problem.

Reference computation (per molecule, B=32, N=512, D=1024, H=512):
  y  = sum_atoms(mask * (ssp(rep @ W1 + b1) @ W2 + b2))           atomwise MLP + pool
  q  = ssp(rep @ Wc1 + bc1) @ Wc2 + bc2                           charge net
  e  = sum_{i!=j} q_i q_j (1e-5 + |R_i - R_j|)^-2 * mask_i mask_j coulomb term
  out = y + e

Sharding: data-parallel over molecules, 4 molecules per core on 8 cores,
weights replicated. Per core everything is computed transposed
(z^T = W^T @ rep^T) so biases are per-partition and the second-layer
contraction over H runs on the PE with h on partitions.

ssp(x) = softplus(x) - ln2 is folded as softplus on device plus a host-side
constant shift c = b - ln2 * sum(W_layer2) applied at the pooled level.

Pairwise distances are computed exactly (per coordinate broadcast-subtract +
square on ACT, accumulate on DVE) rather than via the |ri|^2+|rj|^2-2ri.rj
matmul trick, which loses ~3 decimal digits to cancellation for close pairs.
"""

import numpy as np

import concourse.bass as bass
import concourse.bacc as bacc
import concourse.mybir as mybir
import concourse.tile as tile
from concourse import bass_utils
from concourse.masks import make_identity

LOG2 = float(np.log(2.0))

B, N, D, H = 32, 512, 1024, 512
NCORES = 8
BL = B // NCORES          # molecules per core
P = 128                   # partitions
KD = D // P               # 8 K-chunks over D
HC = H // P               # 4 h-chunks over H
IC = N // P               # 4 i-chunks over atoms

f32 = mybir.dt.float32
f32r = mybir.dt.float32r
AF = mybir.ActivationFunctionType
ALU = mybir.AluOpType
AX = mybir.AxisListType

_CACHE = {}

# Every ACT function this kernel uses (Exp, Ln, Square, Copy, Identity) lives
# in the "natural_log_exp_and_others" table set. Bacc's table chooser is
# greedy-first-match, which makes alternating Exp/Ln streams flip-flop between
# "exp_and_others" and "natural_log" — 65 table loads x ~2.7us. Emptying every
# other set (order preserved, so act_func_set_id indices stay valid) pins the
# chooser to the combined set: one load for the whole kernel.
_ONE_TABLE = "natural_log_exp_and_others"


def _gat_one_table(arch):
    from concourse.hw_specs import get_activation_tables
    tabs = get_activation_tables(arch)
    assert _ONE_TABLE in tabs
    return {n: (fns if n == _ONE_TABLE else set()) for n, fns in tabs.items()}


def _build_program():
    bacc.get_activation_tables = _gat_one_table
    nc = bacc.Bacc("TRN2", target_bir_lowering=False, debug=False,
                   enable_asserts=False)

    rep_d = nc.dram_tensor("rep", [BL * N, D], f32r, kind="ExternalInput").ap()
    w1_d = nc.dram_tensor("w1", [D, H], f32r, kind="ExternalInput").ap()
    wc1_d = nc.dram_tensor("wc1", [D, H], f32r, kind="ExternalInput").ap()
    b1t_d = nc.dram_tensor("b1t", [P, HC], f32, kind="ExternalInput").ap()
    bc1t_d = nc.dram_tensor("bc1t", [P, HC], f32, kind="ExternalInput").ap()
    w2t_d = nc.dram_tensor("w2t", [P, HC], f32r, kind="ExternalInput").ap()
    wc2t_d = nc.dram_tensor("wc2t", [P, HC], f32r, kind="ExternalInput").ap()
    rrows_d = nc.dram_tensor("rrows", [BL, 3, N], f32, kind="ExternalInput").ap()
    rcoln_d = nc.dram_tensor("rcoln", [P, BL * IC * 3], f32, kind="ExternalInput").ap()
    maskr_d = nc.dram_tensor("maskr", [BL, N], f32, kind="ExternalInput").ap()
    cvec_d = nc.dram_tensor("cvec", [1, BL + 1], f32, kind="ExternalInput").ap()
    out_d = nc.dram_tensor("out", [1, BL], f32, kind="ExternalOutput").ap()

    with tile.TileContext(nc) as tc:
        with tc.tile_pool(name="singles", bufs=1) as singles, \
             tc.tile_pool(name="work", bufs=1) as work, \
             tc.tile_pool(name="ps", bufs=1, space="PSUM") as ps:

            ident = singles.tile([P, P], f32, tag="ident")
            make_identity(nc, ident)
            identr = singles.tile([P, P], f32r, tag="identr")
            nc.vector.tensor_copy(identr, ident)
            ident32 = singles.tile([1, 1], f32, tag="ident32")
            nc.vector.memset(ident32, 1.0)
            ones_col = singles.tile([P, 1], f32, tag="ones_col")
            nc.vector.memset(ones_col, 1.0)
            floor_col = singles.tile([P, 1], f32, tag="floor_col")
            nc.vector.memset(floor_col, 1e-30)

            # rep tiles stream on both HWDGE rings (SP + ACT), issued first so
            # the PE transposes can start a few us in; everything else rides
            # the eight SWDGE queues (gpsimd) concurrently.
            rcoln = singles.tile([P, BL * IC * 3], f32, tag="rcoln")
            nc.sync.dma_start(rcoln, rcoln_d)
            xjb0 = work.tile([P, 3, N], f32, tag="xjb", bufs=2)
            nc.sync.dma_start(xjb0, rrows_d[0].partition_broadcast(P))
            repm_all = []

            def load_rep(b):
                for mb in range(IC):
                    t = work.tile([P, D], f32r, tag="repm", bufs=2 * IC)
                    eng = nc.sync if (mb % 2 == 0) else nc.scalar
                    eng.dma_start(
                        t, rep_d[b * N + mb * P: b * N + (mb + 1) * P, :])
                    repm_all.append(t)

            # molecule-0 rep first, then the first two weight K-chunks on the
            # fast rings (the first z matmuls need them), then the rest
            load_rep(0)
            w1_sb = [None] * KD
            wc1_sb = [None] * KD
            for k in range(KD):
                w1_sb[k] = singles.tile([P, H], f32r, tag=f"w1_{k}", name=f"w1sb{k}")
                wc1_sb[k] = singles.tile([P, H], f32r, tag=f"wc1_{k}", name=f"wc1sb{k}")
            for k in (0, 1):
                eng = nc.sync if k == 0 else nc.scalar
                eng.dma_start(w1_sb[k], w1_d[k * P:(k + 1) * P, :])
                eng.dma_start(wc1_sb[k], wc1_d[k * P:(k + 1) * P, :])
            for b in range(1, BL):
                load_rep(b)
            for k in range(2, KD):
                nc.gpsimd.dma_start(w1_sb[k], w1_d[k * P:(k + 1) * P, :])
                nc.gpsimd.dma_start(wc1_sb[k], wc1_d[k * P:(k + 1) * P, :])

            b1t = singles.tile([P, HC], f32, tag="b1t")
            nc.gpsimd.dma_start(b1t, b1t_d)
            bc1t = singles.tile([P, HC], f32, tag="bc1t")
            nc.gpsimd.dma_start(bc1t, bc1t_d)
            w2t = singles.tile([P, HC], f32r, tag="w2t")
            nc.gpsimd.dma_start(w2t, w2t_d)
            wc2t = singles.tile([P, HC], f32r, tag="wc2t")
            nc.gpsimd.dma_start(wc2t, wc2t_d)
            cvec = singles.tile([1, BL + 1], f32, tag="cvec")
            nc.gpsimd.dma_start(cvec, cvec_d)
            mrows = []
            for b in range(BL):
                m = singles.tile([1, N], f32, tag=f"mrow_{b}")
                nc.gpsimd.dma_start(m, maskr_d[b:b + 1, :])
                mrows.append(m)
            res = singles.tile([1, BL], f32, tag="res")

            # ---- pairwise chain: rb[p, ic, j] = (1e-5 + d_(128ic+p),j)^-2 ----
            # All four 128-row i-chunks are packed in one [P, IC, N] tile so
            # each elementwise stage is a single large instruction.
            # d2[p, ic, j] = |R_(128*ic+p) - R_j|^2, built from one ACT Square
            # per coordinate per chunk (bias = -coord_i), then
            # r = (1e-5 + sqrt(d2))^-2 with sqrt = exp(0.5*ln(d2)) to stay on
            # the single exp/ln ACT table set (a table switch costs ~2.7us).
            # d2 is exactly 0 on each chunk's diagonal; the 1e-30 ln floor
            # keeps it finite there (off-diagonal d2 >= ~1e-4 so the floor is
            # invisible) and the fused affine_select zeroes all diagonals.
            # The chain has no PE work and no dependency on the same
            # molecule's MLP, so it is emitted one molecule AHEAD: it fills
            # ACT/DVE/GpSimd while the PE grinds the previous molecule's
            # matmuls, and rb is ready when the t-matvec needs it.
            def emit_chain(b):
                if b == 0:
                    xjb = xjb0
                else:
                    xjb = work.tile([P, 3, N], f32, tag="xjb", bufs=2)
                    nc.gpsimd.dma_start(xjb, rrows_d[b].partition_broadcast(P))
                d2b = work.tile([P, IC, N], f32, tag="d2b", bufs=1)
                tmpb = work.tile([P, IC, N], f32, tag="tmpb", bufs=1)
                for ic in range(IC):
                    col = (b * IC + ic) * 3
                    nc.scalar.activation(d2b[:, ic, :], xjb[:, 0, :], AF.Square,
                                         bias=rcoln[:, col + 0:col + 1])
                    nc.scalar.activation(tmpb[:, ic, :], xjb[:, 1, :], AF.Square,
                                         bias=rcoln[:, col + 1:col + 2])
                nc.vector.tensor_add(d2b, d2b, tmpb)
                for ic in range(IC):
                    col = (b * IC + ic) * 3
                    nc.scalar.activation(tmpb[:, ic, :], xjb[:, 2, :], AF.Square,
                                         bias=rcoln[:, col + 2:col + 3])
                nc.vector.tensor_add(d2b, d2b, tmpb)
                nc.scalar.activation(d2b, d2b, AF.Ln, bias=floor_col[:, 0:1])
                nc.scalar.activation(d2b, d2b, AF.Exp, scale=0.5)
                nc.gpsimd.tensor_scalar(d2b, d2b, 1e-5, None, op0=ALU.add)
                rcb = work.tile([P, IC, N], f32, tag="rcb", bufs=1)
                nc.vector.reciprocal(rcb, d2b)
                rb = work.tile([P, IC, N], f32r, tag="rb", bufs=2)
                nc.vector.tensor_mul(rb, rcb, rcb)
                # zero the diagonal of every chunk: j == p + 128*ic
                nc.gpsimd.affine_select(
                    out=rb, in_=rb, compare_op=ALU.not_equal, fill=0.0,
                    base=0, pattern=[[P, IC], [-1, N]], channel_multiplier=1)
                return rb

            rb_tiles = {0: emit_chain(0)}

            for b in range(BL):
                repm = repm_all[b * IC:(b + 1) * IC]
                rT = []
                for db in range(KD):
                    tp = ps.tile([P, N], f32r, tag="tp", bufs=3)
                    for mb in range(IC):
                        nc.tensor.transpose(
                            tp[:, mb * P:(mb + 1) * P],
                            repm[mb][:, db * P:(db + 1) * P], identr)
                    rt = work.tile([P, N], f32r, tag="rT", bufs=12)
                    nc.vector.tensor_copy(rt, tp)
                    rT.append(rt)

                # ---- MLP layer 1 (transposed): h^T = ssp(W^T @ rep^T + b) ----
                h1T = []
                hqT = []
                for (w_sb, bias, hlist, htag) in (
                        (w1_sb, b1t, h1T, "h1"), (wc1_sb, bc1t, hqT, "hq")):
                    for hc in range(HC):
                        z = ps.tile([P, N], f32, tag="z", bufs=2)
                        for k in range(KD):
                            nc.tensor.matmul(
                                z,
                                lhsT=w_sb[k][:, hc * P:(hc + 1) * P],
                                rhs=rT[k][:],
                                start=(k == 0), stop=(k == KD - 1))
                        # softplus(z + b) = ln(exp(z + b) + 1); Softplus has
                        # no ACT table on this compiler, exp/ln share one set
                        ez = work.tile([P, N], f32, tag="ez", bufs=2)
                        nc.scalar.activation(ez, z, AF.Exp,
                                             bias=bias[:, hc:hc + 1])
                        h = work.tile([P, N], f32r, tag=htag, bufs=2 * HC)
                        nc.scalar.activation(h, ez, AF.Ln, bias=ones_col[:, 0:1])
                        hlist.append(h)

                # prefetch a later molecule's pairwise chain (ACT/DVE/GpSimd
                # only) while this molecule's PE work continues below
                if b + 1 < BL:
                    rb_tiles[b + 1] = emit_chain(b + 1)

                # ---- layer 2 contractions over H on the PE ----
                yi_ps = ps.tile([1, N], f32, tag="row_ps", bufs=2)
                for hc in range(HC):
                    nc.tensor.matmul(yi_ps,
                                     lhsT=w2t[:, hc:hc + 1],
                                     rhs=h1T[hc][:],
                                     start=(hc == 0), stop=(hc == HC - 1))
                q_ps = ps.tile([1, N], f32, tag="row_ps", bufs=2)
                for hc in range(HC):
                    nc.tensor.matmul(q_ps,
                                     lhsT=wc2t[:, hc:hc + 1],
                                     rhs=hqT[hc][:],
                                     start=(hc == 0), stop=(hc == HC - 1))

                mrow = mrows[b]
                # y_b = sum(yi * mask) + cm_b, with cm_b = c2*sum(mask_b)
                # precomputed on host (cvec[0, b]).
                # tensor_tensor_reduce with a PSUM operand faults the exec
                # unit on this runtime; use mul + reduce instead
                scr_y = work.tile([1, N], f32, tag="scr_y", bufs=2)
                nc.vector.tensor_mul(scr_y, yi_ps, mrow)
                ysum = work.tile([1, 1], f32, tag="ysum", bufs=2)
                nc.vector.reduce_sum(ysum, scr_y, axis=AX.X)
                y_sb = work.tile([1, 1], f32, tag="y_sb", bufs=2)
                nc.vector.tensor_add(y_sb, ysum, cvec[0:1, b:b + 1])

                # charge row: qrow = (q + cq) * mask
                qrow = work.tile([1, N], f32, tag="qrow", bufs=2)
                nc.vector.tensor_scalar(qrow, q_ps, cvec[0:1, BL:BL + 1], None,
                                        op0=ALU.add)
                nc.vector.tensor_mul(qrow, qrow, mrow)

                # charge columns (one [128,1] per i-chunk) via PE transpose
                qc_ps = ps.tile([P, IC], f32, tag="qc_ps", bufs=1)
                for ic in range(IC):
                    nc.tensor.transpose(qc_ps[:, ic:ic + 1],
                                        qrow[:, ic * P:(ic + 1) * P],
                                        ident32[0:1, 0:1])
                qc = work.tile([P, IC], f32r, tag="qc", bufs=2)
                nc.scalar.copy(qc, qc_ps)

                rb = rb_tiles.pop(b)
                t_ps = ps.tile([1, N], f32, tag="row_ps", bufs=2)
                for ic in range(IC):
                    nc.tensor.matmul(t_ps,
                                     lhsT=qc[:, ic:ic + 1],
                                     rhs=rb[:, ic, :],
                                     start=(ic == 0), stop=(ic == IC - 1))

                scr_e = work.tile([1, N], f32, tag="scr_e", bufs=2)
                nc.vector.tensor_mul(scr_e, t_ps, qrow)
                e_sb = work.tile([1, 1], f32, tag="e_sb", bufs=2)
                nc.vector.reduce_sum(e_sb, scr_e, axis=AX.X)

                nc.vector.tensor_add(res[:, b:b + 1], y_sb, e_sb)

            nc.sync.dma_start(out_d, res)

    nc.compile()
    return nc


def _get_program():
    if "nc" not in _CACHE:
        _CACHE["nc"] = _build_program()
    return _CACHE["nc"]


def _host_prep(inputs):
    """Build per-core in_maps from full inputs."""
    rep = np.ascontiguousarray(np.asarray(inputs["representation"], np.float32))
    R = np.asarray(inputs["R"], np.float32)
    mask = np.asarray(inputs["atom_mask"], np.float32)
    W1 = np.asarray(inputs["W1"], np.float32)
    b1 = np.asarray(inputs["b1"], np.float32)
    W2 = np.asarray(inputs["W2"], np.float32)
    b2 = np.asarray(inputs["b2"], np.float32)
    Wc1 = np.asarray(inputs["Wc1"], np.float32)
    bc1 = np.asarray(inputs["bc1"], np.float32)
    Wc2 = np.asarray(inputs["Wc2"], np.float32)
    bc2 = np.asarray(inputs["bc2"], np.float32)

    b1t = np.ascontiguousarray(b1.reshape(HC, P).T)
    bc1t = np.ascontiguousarray(bc1.reshape(HC, P).T)
    w2t = np.ascontiguousarray(W2[:, 0].reshape(HC, P).T)
    wc2t = np.ascontiguousarray(Wc2[:, 0].reshape(HC, P).T)
    c2 = np.float32(b2[0] - LOG2 * W2.sum(dtype=np.float64))
    cq = np.float32(bc2[0] - LOG2 * Wc2.sum(dtype=np.float64))

    in_maps = []
    for c in range(NCORES):
        sl = slice(c * BL, (c + 1) * BL)
        Rb = R[sl]                                   # [BL, N, 3]
        rrows = np.ascontiguousarray(Rb.transpose(0, 2, 1))       # [BL,3,N]
        # rcoln[p, (b*IC+ic)*3 + c] = -R[b, ic*128+p, c]
        rcoln = np.ascontiguousarray(
            (-Rb.reshape(BL, IC, P, 3)).transpose(2, 0, 1, 3).reshape(P, BL * IC * 3))
        # cvec = [c2*sum(mask_b) per molecule, then cq]
        cvec = np.concatenate(
            [c2 * mask[sl].sum(axis=1, dtype=np.float32), [cq]]
        ).astype(np.float32).reshape(1, BL + 1)
        in_maps.append({
            "rep": np.ascontiguousarray(rep[sl].reshape(BL * N, D)),
            "w1": W1, "wc1": Wc1,
            "b1t": b1t, "bc1t": bc1t, "w2t": w2t, "wc2t": wc2t,
            "rrows": rrows, "rcoln": rcoln,
            "maskr": np.ascontiguousarray(mask[sl]),
            "cvec": cvec,
        })
    return in_maps


def kernel(**inputs) -> np.ndarray:
    nc = _get_program()
    in_maps = _host_prep(inputs)
    res = None
    last_err = None
    for attempt in range(3):
        try:
            res = bass_utils.run_bass_kernel_spmd(
                nc, in_maps, core_ids=list(range(NCORES)))
            break
        except Exception as e:  # transient NRT_EXEC_UNIT faults have been seen
            last_err = e
            import time
            time.sleep(2.0)
            try:
                import jax
                jax.clear_backends()
            except Exception:
                pass
    if res is None:
        raise last_err
    out = np.concatenate([res.results[c]["out"][0] for c in range(NCORES)])
    return out.reshape(B, 1).astype(np.float32)
